# revision 21
# baseline (speedup 1.0000x reference)
"""GAT single-head forward on 8 Trainium2 NeuronCores (Bass/Tile).

Math (per reference):
    h   = X @ W + b                      [N, 128]
    f1  = h @ v0, f2 = h @ v1            [N]
    logits = adj * (f1[:,None] + f2[None,:])   (adj entries are exactly 0/1)
    vals = sigmoid(logits) - 0.5
    masked softmax over row edges; out = probs @ h

Identities used on device:
  * On edges the softmax weight is w = exp(sigmoid(s)), s = f1_i + f2_j,
    up to a per-row factor that cancels; with t = tanh(s/2):
    w = exp(0.5*t + 0.5). Tanh and Exp share one activation table set.
  * EXP route (exact): w = exp(0.5 t + 0.5), masked by et = A*w (one fused
    tensor_tensor multiply per group -- 2x DVE mode).
  * QUAD route (NQ j-chunks, offloads the ACT-bound exp to the DVE):
    w ~ rho*[(t+d)^2 + C] (minimax rel err 5.4e-3). Realized as
    u = t + 2d (ts 4x), P = u*t (tt 2x), Pc = P + CPQ (ts 4x),
    et = A*Pc (tt 2x), where CPQ = C + d^2. rho is fixed so the two
    routes agree: rho = 1 (weights from either route are used in the SAME
    softmax, so the quad is fit with rho free and then DIVIDED by rho --
    i.e. constants are pre-scaled so both routes approximate exp(...)
    directly).
  * A ones-column appended to h turns the softmax denominator into one
    extra matmul output column.

Sharding: rows of adj across the 8 cores (1024 rows each). node_feats is
replicated; every core computes the full projected h - no collectives.

Per-core layout: adj block TRANSPOSED ([j=source node on partitions,
i=own rows on free dim]) so the aggregate contracts over the partition
dim. adj is cast to fp16 host-side (exact for a 0/1 mask).

Engine budget per core (8.4M dense elements):
  ACT: tanh (all chunks, fused) + exp (chunks not on the quad route)
  DVE: per-q preadd (ts) + per-group fused mask multiplies (tt) + quad ops
  PE : single aggregate et @ h_ext (512 LDW+MM pairs) + h-projection
"""

import os

import numpy as np

import concourse.mybir as mybir
import concourse.tile as tile
from concourse import bacc
from concourse.bass_utils import run_bass_kernel_spmd

F32 = mybir.dt.float32
F16 = mybir.dt.float16
AF = mybir.ActivationFunctionType
ALU = mybir.AluOpType

N, C_IN, C_OUT = 8192, 256, 128
NCORES = 8
ROWS = N // NCORES          # 1024 rows of adj per core
P = 128
NT = N // P                 # 64 node tiles (j-chunks)
NI = ROWS // P              # 8 output row-tiles per core
KC = [128, 128, 1]          # contraction chunks of K=257
WCOLS = C_OUT + 3           # [W | ones-hack | 0.5*w0 | 0.5*w1]
HCOLS = C_OUT + 1           # h plus the ones column
TINY = float(np.finfo(np.float32).tiny)
BANK = 512                  # PSUM bank, fp32 elements

# quadratic approx of exp((1+t)/2) ~ rho*[(t+D)^2 + CC]; both routes feed
# the same softmax, so divide the quad by rho: P/rho family via pre-scaled
# constants. (t+D)^2 + CC = t^2 + 2D t + (D^2+CC); we emit
# u = t/RQ + 2D/RQ... simpler: u = t + 2D, P = u*t, Pc = P*1 + CPQ, then
# et = A*(Pc) and finally weights differ from exp-route by factor rho ->
# fold 1/rho into Pc via PS (see below).
D = 2.079251
CC = 3.749643
RHO = 0.204586                      # fit scale: exp ~= RHO*((t+D)^2+CC)
# y = RHO*((t+2D)*t) ; et = A*(y + RHO*(D*D+CC))
CPQ = float(RHO * (D * D + CC))
# NQ j-chunks take the quad route (engine balance knob; rest take exp)
NQ = 18

GROUPS = [4] + [6] * 9 + [2, 2, 2]  # j-chunk fusion per ACT/DVE instr

_CACHE: dict = {}


def _quad_chunks():
    """Quad-route chunks: the tail groups entirely (so the ACT chain ends
    before the kernel tail), plus a spread through earlier groups for
    ACT/DVE balance."""
    qs = set(range(58, 64))            # the [2,2,2] tail groups
    left = NQ - len(qs)
    q0 = 0
    for gsz in GROUPS[:-3]:
        take = min(gsz, max(0, min(left, 1 + (gsz > 4))))
        for k in range(take):
            qs.add(q0 + k)
        left -= take
        q0 += gsz
    q = 0
    while left > 0 and q < 58:
        if q not in qs:
            qs.add(q); left -= 1
        q += 1
    return qs


QSET = _quad_chunks()


def _build_nc(b_zero=True):
    nc = bacc.Bacc(
        "TRN2", target_bir_lowering=False, debug=False, num_devices=NCORES
    )
    xt1 = nc.dram_tensor("xt1", [257, N], F16, kind="ExternalInput").ap()
    xt1l = nc.dram_tensor("xt1l", [257, ROWS], F16, kind="ExternalInput").ap()
    wext = nc.dram_tensor("wext", [257, WCOLS], F16, kind="ExternalInput").ap()
    adjt = nc.dram_tensor("adjt", [N, ROWS], F16, kind="ExternalInput").ap()
    out = nc.dram_tensor("out", [ROWS, C_OUT], F32, kind="ExternalOutput").ap()

    with tile.TileContext(nc) as tc:
        _emit(tc, nc, xt1, xt1l, wext, adjt, out, b_zero)
    nc.compile()
    return nc


def _emit(tc, nc, xt1, xt1l, wext, adjt, out, b_zero):
    from contextlib import ExitStack

    nkc = 2 if b_zero else 3

    group_q0 = []
    _q0 = 0
    for _g in GROUPS:
        group_q0.append(_q0)
        _q0 += _g

    with ExitStack() as ctx:
        # ---- persistent tiles ----
        persist = ctx.enter_context(tc.tile_pool(name="persist", bufs=1))
        h16_all = persist.tile([P, NT * HCOLS], F16, tag="h16")   # [128, 8256]
        f2h_all = persist.tile([P, NT], F32, tag="f2h")           # 0.5*f2 per j
        f1rep = persist.tile([P, ROWS], F16, tag="f1rep")         # 0.5*f1 bcast
        zero1 = persist.tile([P, 1], F32, tag="zero1")
        nc.vector.memset(zero1[:], 0.0)
        half1 = persist.tile([P, 1], F32, tag="half1")
        nc.vector.memset(half1[:], 0.5)
        if b_zero:
            nc.vector.memset(
                h16_all[:].rearrange("p (t c) -> p t c", c=HCOLS)[
                    :, :, C_OUT : C_OUT + 1
                ],
                1.0,
            )

        xtp = ctx.enter_context(tc.tile_pool(name="xt", bufs=1))
        atp = ctx.enter_context(tc.tile_pool(name="atp", bufs=3))   # adj
        xtp2 = ctx.enter_context(tc.tile_pool(name="xtp2", bufs=2))  # s/2 -> t
        wtp = ctx.enter_context(tc.tile_pool(name="wtp", bufs=2))   # w / quad
        etp = ctx.enter_context(tc.tile_pool(name="etp", bufs=2))   # masked
        obp = ctx.enter_context(tc.tile_pool(name="ob", bufs=2))
        pre_at = {}

        def issue_at_dma(g):
            gsz = GROUPS[g]
            q0 = group_q0[g]
            at_sup = atp.tile([P, gsz * ROWS], F16, tag="at", name=f"at{g}")
            nc.sync.dma_start(
                at_sup[:].rearrange("p (q i) -> p q i", i=ROWS),
                adjt.rearrange("(q p) i -> p q i", p=P)[:, q0 : q0 + gsz, :],
            )
            return at_sup

        # ---- input loads ----
        offs = [0, 128, 256]
        xts = [
            xtp.tile([KC[k], N], F16, name=f"xtsb{k}", tag=f"xt{k}")
            for k in range(nkc)
        ]
        SUBS = [0, 1024, 3072, 5120, N]
        wes, xls = [], []
        off = 0
        for k in range(nkc):
            kc = KC[k]
            wx_sb = xtp.tile([kc, WCOLS + ROWS], F16, name=f"wx{k}", tag=f"wx{k}")
            nc.sync.dma_start(wx_sb[:, 0:WCOLS], wext[off : off + kc, :])
            nc.sync.dma_start(wx_sb[:, WCOLS:], xt1l[off : off + kc, :])
            wes.append(wx_sb[:, 0:WCOLS])
            xls.append(wx_sb[:, WCOLS:])
            off += kc
        for k in range(nkc):
            if KC[k] == P:
                nc.sync.dma_start(
                    xts[k][:, 0 : SUBS[1]],
                    xt1[offs[k] : offs[k] + KC[k], 0 : SUBS[1]],
                )
        pre_at[0] = issue_at_dma(0)
        pre_at[1] = issue_at_dma(1)
        for c in range(1, len(SUBS) - 1):
            for k in range(nkc):
                if KC[k] != P:
                    if c == 1:
                        nc.sync.dma_start(
                            xts[k][:], xt1[offs[k] : offs[k] + KC[k], :]
                        )
                    continue
                nc.sync.dma_start(
                    xts[k][:, SUBS[c] : SUBS[c + 1]],
                    xt1[offs[k] : offs[k] + KC[k], SUBS[c] : SUBS[c + 1]],
                )

        # ---- f1 path ----
        with tc.tile_pool(name="pf", bufs=1, space="PSUM") as pfp:
            prep = pfp.tile([P, ROWS], F32, tag="prep")
            for k in range(nkc):
                for nh in range(ROWS // 512):
                    nc.tensor.matmul(
                        prep[:, nh * 512 : (nh + 1) * 512],
                        wes[k][:, C_OUT + 1 : C_OUT + 2].to_broadcast(
                            (KC[k], P)
                        ),
                        xls[k][:, nh * 512 : (nh + 1) * 512],
                        start=(k == 0),
                        stop=(k == nkc - 1),
                    )
            nc.scalar.copy(f1rep[:], prep[:])

        # ---- f2 head start ----
        F2HEAD = 8
        with tc.tile_pool(name="pf2", bufs=1, space="PSUM") as pf2p:
            pt = pf2p.tile([P, NI * BANK], F32, tag="pt")
            pt3 = pt[:].rearrange("p (t w) -> p t w", w=BANK)
            for q in range(F2HEAD):
                w = (q % NI) * BANK
                for k in range(nkc):
                    nc.tensor.matmul(
                        pt[:, w : w + 1],
                        xts[k][:, q * P : (q + 1) * P],
                        wes[k][:, C_OUT + 2 : C_OUT + 3],
                        start=(k == 0),
                        stop=(k == nkc - 1),
                    )
                if q == 1:
                    nc.vector.tensor_copy(
                        f2h_all[:, 0:2], pt3[:, 0:2, 0:1]
                    )
            nc.vector.tensor_copy(
                f2h_all[:, 2:F2HEAD], pt3[:, 2:F2HEAD, 0:1]
            )

        fa_list = []   # stage-A done (tiles through exp)
        fb_list = []   # stage-B done (masked et ready)

        def emit_group_a(g):
            """adj DMA, per-q preadds (DVE), fused tanh + exp runs (ACT)."""
            gsz = GROUPS[g]
            q0 = group_q0[g]
            at_sup = pre_at.pop(g) if g in pre_at else issue_at_dma(g)
            xg = xtp2.tile([P, gsz * ROWS], F16, tag="xg", name=f"xg{g}")
            for qq in range(gsz):
                q = q0 + qq
                nc.vector.tensor_scalar_add(
                    xg[:, qq * ROWS : (qq + 1) * ROWS],
                    f1rep[:],
                    f2h_all[:, q : q + 1],
                )
            # t = tanh(s/2) in place (proven reader-rewriter pattern)
            nc.scalar.activation(xg[:], xg[:], AF.Tanh, bias=zero1[:])
            wg = wtp.tile([P, gsz * ROWS], F16, tag="wg", name=f"wg{g}")
            runs = []  # (start_qq, end_qq, is_quad)
            for qq in range(gsz):
                isq = (q0 + qq) in QSET
                if runs and runs[-1][2] == isq:
                    runs[-1][1] = qq + 1
                else:
                    runs.append([qq, qq + 1, isq])
            for r0, r1, isq in runs:
                sl = slice(r0 * ROWS, r1 * ROWS)
                if not isq:
                    # exact route: w = exp(0.5 t + 0.5)
                    nc.scalar.activation(
                        wg[:, sl], xg[:, sl], AF.Exp, bias=half1[:], scale=0.5
                    )
            return {"g": g, "gsz": gsz, "q0": q0, "at": at_sup,
                    "xg": xg, "wg": wg, "runs": runs}

        def emit_group_b(fr):
            """quad-route DVE ops + mask multiplies."""
            gsz, q0 = fr["gsz"], fr["q0"]
            at_sup, xg, wg, runs = fr["at"], fr["xg"], fr["wg"], fr["runs"]
            et = etp.tile([P, gsz * ROWS], F16, tag="et", name=f"et{fr['g']}")
            for r0, r1, isq in runs:
                sl = slice(r0 * ROWS, r1 * ROWS)
                if isq:
                    # quad: u = RHO*t + 2*D*RHO (ts 4x), y = u*t (tt 2x),
                    # et = (y + CPQ)*A (stt, single writer per range)
                    nc.vector.tensor_scalar(
                        wg[:, sl], xg[:, sl],
                        float(RHO), float(2 * D * RHO), ALU.mult, ALU.add,
                    )
                    yq = wtp.tile(
                        [P, (r1 - r0) * ROWS], F16, tag="yq", name=f"yq{fr['g']}"
                    )
                    nc.vector.tensor_mul(yq[:], wg[:, sl], xg[:, sl])
                    yq2 = wtp.tile(
                        [P, (r1 - r0) * ROWS], F16, tag="yq2",
                        name=f"yq2{fr['g']}"
                    )
                    nc.vector.tensor_scalar_add(yq2[:], yq[:], CPQ)
                    nc.vector.tensor_mul(et[:, sl], at_sup[:, sl], yq2[:])
                else:
                    # exact route mask: et = A * w
                    nc.vector.tensor_mul(
                        et[:, sl], at_sup[:, sl], wg[:, sl]
                    )
            return {"g": fr["g"], "gsz": gsz, "q0": q0, "et": et}

        def emit_group_back(fr, po_all, mid_a=None, mid_b=None):
            gsz, q0, et = fr["gsz"], fr["q0"], fr["et"]
            for qq in range(gsz):
                if qq == min(1, gsz - 1) and mid_a is not None:
                    mid_a()
                if qq == min(3, gsz - 1) and mid_b is not None:
                    mid_b()
                q = q0 + qq
                rhs = h16_all[:, q * HCOLS : (q + 1) * HCOLS]
                for it in range(NI):
                    nc.tensor.matmul(
                        po_all[:, it * BANK : it * BANK + HCOLS],
                        et[:, qq * ROWS + it * P : qq * ROWS + (it + 1) * P],
                        rhs,
                        start=(q == 0),
                        stop=(q == NT - 1),
                    )

        # ---- h-projection ----
        next_group = 0
        with tc.tile_pool(name="php", bufs=1, space="PSUM") as php:
            ph_all = php.tile([P, NI * BANK], F32, tag="ph")
            for b in range(NT // 4):
                for half in range(2):
                    nt0 = 4 * b + 2 * half
                    w0 = (nt0 % NI) * BANK
                    w1 = ((nt0 + 1) % NI) * BANK
                    for k in range(nkc):
                        nc.tensor.matmul(
                            ph_all[:, w0 : w0 + WCOLS],
                            xts[k][:, nt0 * P : (nt0 + 1) * P],
                            wes[k][:],
                            start=(k == 0),
                            stop=(k == nkc - 1),
                        )
                        nc.tensor.matmul(
                            ph_all[:, w1 : w1 + WCOLS],
                            xts[k][:, (nt0 + 1) * P : (nt0 + 2) * P],
                            wes[k][:],
                            start=(k == 0),
                            stop=(k == nkc - 1),
                        )
                bt = 4 * b
                wlo = (bt % NI) * BANK
                src = ph_all[:, wlo : wlo + 4 * BANK].rearrange(
                    "p (b w) -> p b w", b=4
                )
                dst_h = h16_all[:, bt * HCOLS : (bt + 4) * HCOLS].rearrange(
                    "p (b w) -> p b w", b=4
                )
                hc = C_OUT if b_zero else HCOLS
                nc.vector.tensor_copy(dst_h[:, :, 0:hc], src[:, :, 0:hc])
                if bt >= 8:
                    nc.vector.tensor_copy(
                        f2h_all[:, bt : bt + 4],
                        src[:, :, C_OUT + 2 : C_OUT + 3],
                    )
                while (
                    next_group < len(GROUPS)
                    and group_q0[next_group] + GROUPS[next_group] <= 4 * (b + 1)
                    and len(fa_list) + len(fb_list) < 2
                ):
                    fa_list.append(emit_group_a(next_group))
                    next_group += 1
                if len(fa_list) >= 2 and not fb_list:
                    fb_list.append(emit_group_b(fa_list.pop(0)))

        # ---- aggregate accumulators ----
        pop = ctx.enter_context(tc.tile_pool(name="po", bufs=1, space="PSUM"))
        po_all = pop.tile([P, NI * BANK], F32, tag="poall")

        # steady pipeline: back(g) mid-emits stage-A(g+2) then stage-B(g+1)
        def advance_a():
            nonlocal next_group
            if next_group < len(GROUPS):
                fa_list.append(emit_group_a(next_group))
                next_group += 1

        def advance_b():
            if fa_list:
                fb_list.append(emit_group_b(fa_list.pop(0)))

        while not fb_list:
            if not fa_list:
                advance_a()
            advance_b()
        while fb_list:
            fr = fb_list.pop(0)
            emit_group_back(fr, po_all, mid_a=advance_a, mid_b=advance_b)

        # ---- epilogue ----
        ns = obp.tile([P, NI * HCOLS], F32, tag="ns")
        ns3 = ns[:].rearrange("p (t c) -> p t c", c=HCOLS)
        dm = obp.tile([P, NI], F32, tag="dm")
        for it in range(NI):
            if it % 2 == 0:
                nc.vector.tensor_copy(
                    ns3[:, it, :], po_all[:, it * BANK : it * BANK + HCOLS]
                )
            else:
                nc.scalar.copy(
                    ns3[:, it, :], po_all[:, it * BANK : it * BANK + HCOLS]
                )
            nc.vector.tensor_scalar_max(
                dm[:, it : it + 1], ns3[:, it, C_OUT : C_OUT + 1], TINY
            )
        rc = obp.tile([P, NI], F32, tag="rc")
        nc.vector.reciprocal(rc[:], dm[:])
        ob_all = obp.tile([P, NI * C_OUT], F32, tag="oball")
        for it in range(NI):
            if it % 2 == 0:
                nc.vector.tensor_scalar_mul(
                    ob_all[:, it * C_OUT : (it + 1) * C_OUT],
                    ns3[:, it, 0:C_OUT],
                    rc[:, it : it + 1],
                )
            else:
                nc.scalar.mul(
                    ob_all[:, it * C_OUT : (it + 1) * C_OUT],
                    ns3[:, it, 0:C_OUT],
                    rc[:, it : it + 1],
                )
        nc.sync.dma_start(
            out.rearrange("(t p) c -> p t c", p=P),
            ob_all[:].rearrange("p (t c) -> p t c", c=C_OUT),
        )


def _prep_inputs(node_feats, adj_matrix, W, b, v0, v1):
    X = np.ascontiguousarray(node_feats, dtype=np.float32)
    W = np.asarray(W, dtype=np.float32)
    b = np.asarray(b, dtype=np.float32)
    v0 = np.asarray(v0, dtype=np.float32)
    v1 = np.asarray(v1, dtype=np.float32)

    w0h = (0.5 * (W.astype(np.float64) @ v0.astype(np.float64))).astype(np.float32)
    w1h = (0.5 * (W.astype(np.float64) @ v1.astype(np.float64))).astype(np.float32)
    c0h = np.float32(0.5 * float(b.astype(np.float64) @ v0.astype(np.float64)))
    c1h = np.float32(0.5 * float(b.astype(np.float64) @ v1.astype(np.float64)))

    XT1 = np.empty((257, N), np.float32)
    XT1[:256] = X.T
    XT1[256] = 1.0

    WE = np.zeros((257, WCOLS), np.float32)
    WE[:256, :C_OUT] = W
    WE[256, :C_OUT] = b
    WE[256, C_OUT] = 1.0
    WE[:256, C_OUT + 1] = w0h
    WE[256, C_OUT + 1] = c0h
    WE[:256, C_OUT + 2] = w1h
    WE[256, C_OUT + 2] = c1h

    XT1h = XT1.astype(np.float16)
    WEh = WE.astype(np.float16)
    A16 = np.asarray(adj_matrix, dtype=np.float16)

    in_maps = []
    for c in range(NCORES):
        in_maps.append(
            {
                "xt1": XT1h,
                "xt1l": np.ascontiguousarray(XT1h[:, c * ROWS : (c + 1) * ROWS]),
                "wext": WEh,
                "adjt": np.ascontiguousarray(
                    A16[c * ROWS : (c + 1) * ROWS, :].T
                ),
            }
        )
    return in_maps


def _run(in_maps, trace=False, b_zero=True):
    key = f"nc_b{int(b_zero)}"
    if key not in _CACHE:
        _CACHE[key] = _build_nc(b_zero=b_zero)
    nc = _CACHE[key]
    res = run_bass_kernel_spmd(
        nc, in_maps, core_ids=list(range(NCORES)), trace=trace
    )
    full = np.concatenate(
        [res.results[c]["out"] for c in range(NCORES)], axis=0
    ).astype(np.float32)
    return full, res


def kernel(node_feats, adj_matrix, W, b, v0, v1):
    in_maps = _prep_inputs(node_feats, adj_matrix, W, b, v0, v1)
    trace = bool(int(os.environ.get("GAT_TRACE", "0")))
    b_zero = not bool(np.any(np.asarray(b)))
    full, _ = _run(in_maps, trace=trace, b_zero=b_zero)
    return full


# revision 29
# speedup vs baseline: 1.0702x; 1.0702x over previous
"""GAT single-head forward on 8 Trainium2 NeuronCores (Bass/Tile).

Math (per reference):
    h   = X @ W + b                      [N, 128]
    f1  = h @ v0, f2 = h @ v1            [N]
    logits = adj * (f1[:,None] + f2[None,:])   (adj entries are exactly 0/1)
    vals = sigmoid(logits) - 0.5
    masked softmax over row edges; out = probs @ h

Identities used on device:
  * On edges the softmax weight is w = exp(sigmoid(s)), s = f1_i + f2_j,
    up to a per-row factor that cancels; with t = tanh(s/2):
    w = exp(0.5*t + 0.5). Tanh and Exp share one activation table set.
  * EXP route (exact): w = exp(0.5 t + 0.5), masked by et = A*w (one fused
    tensor_tensor multiply per group -- 2x DVE mode).
  * QUAD route (NQ j-chunks, offloads the ACT-bound exp to the DVE):
    w ~ rho*[(t+d)^2 + C] (minimax rel err 5.4e-3). Realized as
    u = t + 2d (ts 4x), P = u*t (tt 2x), Pc = P + CPQ (ts 4x),
    et = A*Pc (tt 2x), where CPQ = C + d^2. rho is fixed so the two
    routes agree: rho = 1 (weights from either route are used in the SAME
    softmax, so the quad is fit with rho free and then DIVIDED by rho --
    i.e. constants are pre-scaled so both routes approximate exp(...)
    directly).
  * A ones-column appended to h turns the softmax denominator into one
    extra matmul output column.

Sharding: rows of adj across the 8 cores (1024 rows each). node_feats is
replicated; every core computes the full projected h - no collectives.

Per-core layout: adj block TRANSPOSED ([j=source node on partitions,
i=own rows on free dim]) so the aggregate contracts over the partition
dim. adj is cast to fp16 host-side (exact for a 0/1 mask).

Engine budget per core (8.4M dense elements):
  ACT: tanh (all chunks, fused) + exp (chunks not on the quad route)
  DVE: per-q preadd (ts) + per-group fused mask multiplies (tt) + quad ops
  PE : single aggregate et @ h_ext (512 LDW+MM pairs) + h-projection
"""

import os

import numpy as np

import concourse.mybir as mybir
import concourse.tile as tile
from concourse import bacc
from concourse.bass_utils import run_bass_kernel_spmd

F32 = mybir.dt.float32
F16 = mybir.dt.float16
AF = mybir.ActivationFunctionType
ALU = mybir.AluOpType

N, C_IN, C_OUT = 8192, 256, 128
NCORES = 8
ROWS = N // NCORES          # 1024 rows of adj per core
P = 128
NT = N // P                 # 64 node tiles (j-chunks)
NI = ROWS // P              # 8 output row-tiles per core
KC = [128, 128, 1]          # contraction chunks of K=257
WCOLS = C_OUT + 3           # [W | ones-hack | 0.5*w0 | 0.5*w1]
HCOLS = C_OUT + 1           # h plus the ones column
TINY = float(np.finfo(np.float32).tiny)
BANK = 512                  # PSUM bank, fp32 elements

# quadratic approx of exp((1+t)/2) ~ rho*[(t+D)^2 + CC]; both routes feed
# the same softmax, so divide the quad by rho: P/rho family via pre-scaled
# constants. (t+D)^2 + CC = t^2 + 2D t + (D^2+CC); we emit
# u = t/RQ + 2D/RQ... simpler: u = t + 2D, P = u*t, Pc = P*1 + CPQ, then
# et = A*(Pc) and finally weights differ from exp-route by factor rho ->
# fold 1/rho into Pc via PS (see below).
D = 2.079251
CC = 3.749643
RHO = 0.204586                      # fit scale: exp ~= RHO*((t+D)^2+CC)
# y = RHO*((t+2D)*t) ; et = A*(y + RHO*(D*D+CC))
CPQ = float(RHO * (D * D + CC))
# NQ j-chunks take the quad route (engine balance knob; rest take exp)
NQ = 20

GROUPS = [4] * 13 + [2] * 6         # j-chunk fusion per ACT/DVE instr

_CACHE: dict = {}


def _quad_chunks():
    """Quad-route chunks: alternate tail groups (so ACT and DVE finish the
    kernel together) plus a spread through earlier groups for balance."""
    qs = set()
    for g in (14, 16, 18):             # [2]-groups -> chunks 54-55,58-59,62-63
        q0 = sum(GROUPS[:g])
        qs.update(range(q0, q0 + GROUPS[g]))
    left = NQ - len(qs)
    q0 = 0
    for gi, gsz in enumerate(GROUPS[:13]):
        if left <= 0:
            break
        take = min(gsz, left, 1)
        for k in range(take):
            qs.add(q0 + k)
        left -= take
        q0 += gsz
    q = 0
    while left > 0 and q < 52:
        if q not in qs:
            qs.add(q); left -= 1
        q += 1
    return qs


QSET = _quad_chunks()


def _build_nc(b_zero=True):
    nc = bacc.Bacc(
        "TRN2", target_bir_lowering=False, debug=False, num_devices=NCORES
    )
    xt1 = nc.dram_tensor("xt1", [257, N], F16, kind="ExternalInput").ap()
    xt1l = nc.dram_tensor("xt1l", [257, ROWS], F16, kind="ExternalInput").ap()
    wext = nc.dram_tensor("wext", [257, WCOLS], F16, kind="ExternalInput").ap()
    adjt = nc.dram_tensor("adjt", [N, ROWS], F16, kind="ExternalInput").ap()
    out = nc.dram_tensor("out", [ROWS, C_OUT], F32, kind="ExternalOutput").ap()

    with tile.TileContext(nc) as tc:
        _emit(tc, nc, xt1, xt1l, wext, adjt, out, b_zero)
    nc.compile()
    return nc


def _emit(tc, nc, xt1, xt1l, wext, adjt, out, b_zero):
    from contextlib import ExitStack

    nkc = 2 if b_zero else 3

    group_q0 = []
    _q0 = 0
    for _g in GROUPS:
        group_q0.append(_q0)
        _q0 += _g

    with ExitStack() as ctx:
        # ---- persistent tiles ----
        persist = ctx.enter_context(tc.tile_pool(name="persist", bufs=1))
        h16_all = persist.tile([P, NT * HCOLS], F16, tag="h16")   # [128, 8256]
        f2h_all = persist.tile([P, NT], F32, tag="f2h")           # 0.5*f2 per j
        f1rep = persist.tile([P, ROWS], F16, tag="f1rep")         # 0.5*f1 bcast
        zero1 = persist.tile([P, 1], F32, tag="zero1")
        nc.vector.memset(zero1[:], 0.0)
        half1 = persist.tile([P, 1], F32, tag="half1")
        nc.vector.memset(half1[:], 0.5)
        if b_zero:
            nc.vector.memset(
                h16_all[:].rearrange("p (t c) -> p t c", c=HCOLS)[
                    :, :, C_OUT : C_OUT + 1
                ],
                1.0,
            )

        xtp = ctx.enter_context(tc.tile_pool(name="xt", bufs=1))
        atp = ctx.enter_context(tc.tile_pool(name="atp", bufs=4))   # adj
        xtp2 = ctx.enter_context(tc.tile_pool(name="xtp2", bufs=3))  # s/2 -> t
        wtp = ctx.enter_context(tc.tile_pool(name="wtp", bufs=3))   # w / quad
        etp = ctx.enter_context(tc.tile_pool(name="etp", bufs=3))   # masked
        obp = ctx.enter_context(tc.tile_pool(name="ob", bufs=2))
        pre_at = {}

        def issue_at_dma(g):
            gsz = GROUPS[g]
            q0 = group_q0[g]
            at_sup = atp.tile([P, gsz * ROWS], F16, tag="at", name=f"at{g}")
            nc.sync.dma_start(
                at_sup[:].rearrange("p (q i) -> p q i", i=ROWS),
                adjt.rearrange("(q p) i -> p q i", p=P)[:, q0 : q0 + gsz, :],
            )
            return at_sup

        # ---- input loads ----
        offs = [0, 128, 256]
        xts = [
            xtp.tile([KC[k], N], F16, name=f"xtsb{k}", tag=f"xt{k}")
            for k in range(nkc)
        ]
        SUBS = [0, 1024, 3072, 5120, N]
        wes, xls = [], []
        off = 0
        for k in range(nkc):
            kc = KC[k]
            wx_sb = xtp.tile([kc, WCOLS + ROWS], F16, name=f"wx{k}", tag=f"wx{k}")
            nc.sync.dma_start(wx_sb[:, 0:WCOLS], wext[off : off + kc, :])
            nc.sync.dma_start(wx_sb[:, WCOLS:], xt1l[off : off + kc, :])
            wes.append(wx_sb[:, 0:WCOLS])
            xls.append(wx_sb[:, WCOLS:])
            off += kc
        for k in range(nkc):
            if KC[k] == P:
                nc.sync.dma_start(
                    xts[k][:, 0 : SUBS[1]],
                    xt1[offs[k] : offs[k] + KC[k], 0 : SUBS[1]],
                )
        pre_at[0] = issue_at_dma(0)
        pre_at[1] = issue_at_dma(1)
        pre_at[2] = issue_at_dma(2)
        for c in range(1, len(SUBS) - 1):
            for k in range(nkc):
                if KC[k] != P:
                    if c == 1:
                        nc.sync.dma_start(
                            xts[k][:], xt1[offs[k] : offs[k] + KC[k], :]
                        )
                    continue
                nc.sync.dma_start(
                    xts[k][:, SUBS[c] : SUBS[c + 1]],
                    xt1[offs[k] : offs[k] + KC[k], SUBS[c] : SUBS[c + 1]],
                )

        # ---- f1 path ----
        with tc.tile_pool(name="pf", bufs=1, space="PSUM") as pfp:
            prep = pfp.tile([P, ROWS], F32, tag="prep")
            for k in range(nkc):
                for nh in range(ROWS // 512):
                    nc.tensor.matmul(
                        prep[:, nh * 512 : (nh + 1) * 512],
                        wes[k][:, C_OUT + 1 : C_OUT + 2].to_broadcast(
                            (KC[k], P)
                        ),
                        xls[k][:, nh * 512 : (nh + 1) * 512],
                        start=(k == 0),
                        stop=(k == nkc - 1),
                    )
            nc.scalar.copy(f1rep[:], prep[:])

        # ---- f2 head start ----
        F2HEAD = 8
        with tc.tile_pool(name="pf2", bufs=1, space="PSUM") as pf2p:
            pt = pf2p.tile([P, NI * BANK], F32, tag="pt")
            pt3 = pt[:].rearrange("p (t w) -> p t w", w=BANK)
            for q in range(F2HEAD):
                w = (q % NI) * BANK
                for k in range(nkc):
                    nc.tensor.matmul(
                        pt[:, w : w + 1],
                        xts[k][:, q * P : (q + 1) * P],
                        wes[k][:, C_OUT + 2 : C_OUT + 3],
                        start=(k == 0),
                        stop=(k == nkc - 1),
                    )
                if q == 1:
                    nc.vector.tensor_copy(
                        f2h_all[:, 0:2], pt3[:, 0:2, 0:1]
                    )
            nc.vector.tensor_copy(
                f2h_all[:, 2:F2HEAD], pt3[:, 2:F2HEAD, 0:1]
            )

        fa_list = []   # stage-A done (tiles through exp)
        fb_list = []   # stage-B done (masked et ready)

        def emit_group_a(g):
            """adj DMA, per-q preadds (DVE), fused tanh + exp runs (ACT)."""
            gsz = GROUPS[g]
            q0 = group_q0[g]
            at_sup = pre_at.pop(g) if g in pre_at else issue_at_dma(g)
            xg = xtp2.tile([P, gsz * ROWS], F16, tag="xg", name=f"xg{g}")
            for qq in range(gsz):
                q = q0 + qq
                nc.vector.tensor_scalar_add(
                    xg[:, qq * ROWS : (qq + 1) * ROWS],
                    f1rep[:],
                    f2h_all[:, q : q + 1],
                )
            # t = tanh(s/2) in place (proven reader-rewriter pattern)
            nc.scalar.activation(xg[:], xg[:], AF.Tanh, bias=zero1[:])
            wg = wtp.tile([P, gsz * ROWS], F16, tag="wg", name=f"wg{g}")
            runs = []  # (start_qq, end_qq, is_quad)
            for qq in range(gsz):
                isq = (q0 + qq) in QSET
                if runs and runs[-1][2] == isq:
                    runs[-1][1] = qq + 1
                else:
                    runs.append([qq, qq + 1, isq])
            for r0, r1, isq in runs:
                sl = slice(r0 * ROWS, r1 * ROWS)
                if not isq:
                    # exact route: w = exp(0.5 t + 0.5)
                    nc.scalar.activation(
                        wg[:, sl], xg[:, sl], AF.Exp, bias=half1[:], scale=0.5
                    )
            return {"g": g, "gsz": gsz, "q0": q0, "at": at_sup,
                    "xg": xg, "wg": wg, "runs": runs}

        def emit_group_b(fr):
            """quad-route DVE ops + mask multiplies."""
            gsz, q0 = fr["gsz"], fr["q0"]
            at_sup, xg, wg, runs = fr["at"], fr["xg"], fr["wg"], fr["runs"]
            et = etp.tile([P, gsz * ROWS], F16, tag="et", name=f"et{fr['g']}")
            for r0, r1, isq in runs:
                sl = slice(r0 * ROWS, r1 * ROWS)
                if isq:
                    # quad: u = RHO*t + 2*D*RHO (ts 4x), y = u*t (tt 2x),
                    # et = (y + CPQ)*A (stt, single writer per range)
                    nc.vector.tensor_scalar(
                        wg[:, sl], xg[:, sl],
                        float(RHO), float(2 * D * RHO), ALU.mult, ALU.add,
                    )
                    yq = wtp.tile(
                        [P, (r1 - r0) * ROWS], F16, tag="yq", name=f"yq{fr['g']}"
                    )
                    nc.vector.tensor_mul(yq[:], wg[:, sl], xg[:, sl])
                    yq2 = wtp.tile(
                        [P, (r1 - r0) * ROWS], F16, tag="yq2",
                        name=f"yq2{fr['g']}"
                    )
                    nc.vector.tensor_scalar_add(yq2[:], yq[:], CPQ)
                    nc.vector.tensor_mul(et[:, sl], at_sup[:, sl], yq2[:])
                else:
                    # exact route mask: et = A * w
                    nc.vector.tensor_mul(
                        et[:, sl], at_sup[:, sl], wg[:, sl]
                    )
            return {"g": fr["g"], "gsz": gsz, "q0": q0, "et": et}

        def emit_group_back(fr, po_all, mid_a=None, mid_b=None):
            gsz, q0, et = fr["gsz"], fr["q0"], fr["et"]
            for qq in range(gsz):
                if qq == 0 and mid_a is not None:
                    mid_a()
                if qq == min(2, gsz - 1) and mid_b is not None:
                    mid_b()
                q = q0 + qq
                rhs = h16_all[:, q * HCOLS : (q + 1) * HCOLS]
                for it in range(NI):
                    nc.tensor.matmul(
                        po_all[:, it * BANK : it * BANK + HCOLS],
                        et[:, qq * ROWS + it * P : qq * ROWS + (it + 1) * P],
                        rhs,
                        start=(q == 0),
                        stop=(q == NT - 1),
                    )

        # ---- h-projection ----
        next_group = 0
        with tc.tile_pool(name="php", bufs=1, space="PSUM") as php:
            ph_all = php.tile([P, NI * BANK], F32, tag="ph")
            for b in range(NT // 4):
                for half in range(2):
                    nt0 = 4 * b + 2 * half
                    w0 = (nt0 % NI) * BANK
                    w1 = ((nt0 + 1) % NI) * BANK
                    for k in range(nkc):
                        nc.tensor.matmul(
                            ph_all[:, w0 : w0 + WCOLS],
                            xts[k][:, nt0 * P : (nt0 + 1) * P],
                            wes[k][:],
                            start=(k == 0),
                            stop=(k == nkc - 1),
                        )
                        nc.tensor.matmul(
                            ph_all[:, w1 : w1 + WCOLS],
                            xts[k][:, (nt0 + 1) * P : (nt0 + 2) * P],
                            wes[k][:],
                            start=(k == 0),
                            stop=(k == nkc - 1),
                        )
                bt = 4 * b
                wlo = (bt % NI) * BANK
                src = ph_all[:, wlo : wlo + 4 * BANK].rearrange(
                    "p (b w) -> p b w", b=4
                )
                dst_h = h16_all[:, bt * HCOLS : (bt + 4) * HCOLS].rearrange(
                    "p (b w) -> p b w", b=4
                )
                hc = C_OUT if b_zero else HCOLS
                nc.vector.tensor_copy(dst_h[:, :, 0:hc], src[:, :, 0:hc])
                if bt >= 8:
                    nc.vector.tensor_copy(
                        f2h_all[:, bt : bt + 4],
                        src[:, :, C_OUT + 2 : C_OUT + 3],
                    )
                while (
                    next_group < len(GROUPS)
                    and group_q0[next_group] + GROUPS[next_group] <= 4 * (b + 1)
                    and len(fa_list) + len(fb_list) < 3
                ):
                    fa_list.append(emit_group_a(next_group))
                    next_group += 1
                if len(fa_list) >= 2 and not fb_list:
                    fb_list.append(emit_group_b(fa_list.pop(0)))

        # ---- aggregate accumulators ----
        pop = ctx.enter_context(tc.tile_pool(name="po", bufs=1, space="PSUM"))
        po_all = pop.tile([P, NI * BANK], F32, tag="poall")

        # steady pipeline: back(g) mid-emits stage-A(g+2) then stage-B(g+1)
        def advance_a():
            nonlocal next_group
            if next_group < len(GROUPS):
                fa_list.append(emit_group_a(next_group))
                next_group += 1

        def advance_b():
            if fa_list:
                fb_list.append(emit_group_b(fa_list.pop(0)))

        while not fb_list:
            if not fa_list:
                advance_a()
            advance_b()
        while fb_list:
            fr = fb_list.pop(0)
            emit_group_back(fr, po_all, mid_a=advance_a, mid_b=advance_b)

        # ---- epilogue ----
        ns = obp.tile([P, NI * HCOLS], F32, tag="ns")
        ns3 = ns[:].rearrange("p (t c) -> p t c", c=HCOLS)
        dm = obp.tile([P, NI], F32, tag="dm")
        for it in range(NI):
            if it % 2 == 0:
                nc.vector.tensor_copy(
                    ns3[:, it, :], po_all[:, it * BANK : it * BANK + HCOLS]
                )
            else:
                nc.scalar.copy(
                    ns3[:, it, :], po_all[:, it * BANK : it * BANK + HCOLS]
                )
            nc.vector.tensor_scalar_max(
                dm[:, it : it + 1], ns3[:, it, C_OUT : C_OUT + 1], TINY
            )
        rc = obp.tile([P, NI], F32, tag="rc")
        nc.vector.reciprocal(rc[:], dm[:])
        ob_all = obp.tile([P, NI * C_OUT], F32, tag="oball")
        for it in range(NI):
            if it % 2 == 0:
                nc.vector.tensor_scalar_mul(
                    ob_all[:, it * C_OUT : (it + 1) * C_OUT],
                    ns3[:, it, 0:C_OUT],
                    rc[:, it : it + 1],
                )
            else:
                nc.scalar.mul(
                    ob_all[:, it * C_OUT : (it + 1) * C_OUT],
                    ns3[:, it, 0:C_OUT],
                    rc[:, it : it + 1],
                )
        nc.sync.dma_start(
            out.rearrange("(t p) c -> p t c", p=P),
            ob_all[:].rearrange("p (t c) -> p t c", c=C_OUT),
        )


def _prep_inputs(node_feats, adj_matrix, W, b, v0, v1):
    X = np.ascontiguousarray(node_feats, dtype=np.float32)
    W = np.asarray(W, dtype=np.float32)
    b = np.asarray(b, dtype=np.float32)
    v0 = np.asarray(v0, dtype=np.float32)
    v1 = np.asarray(v1, dtype=np.float32)

    w0h = (0.5 * (W.astype(np.float64) @ v0.astype(np.float64))).astype(np.float32)
    w1h = (0.5 * (W.astype(np.float64) @ v1.astype(np.float64))).astype(np.float32)
    c0h = np.float32(0.5 * float(b.astype(np.float64) @ v0.astype(np.float64)))
    c1h = np.float32(0.5 * float(b.astype(np.float64) @ v1.astype(np.float64)))

    XT1 = np.empty((257, N), np.float32)
    XT1[:256] = X.T
    XT1[256] = 1.0

    WE = np.zeros((257, WCOLS), np.float32)
    WE[:256, :C_OUT] = W
    WE[256, :C_OUT] = b
    WE[256, C_OUT] = 1.0
    WE[:256, C_OUT + 1] = w0h
    WE[256, C_OUT + 1] = c0h
    WE[:256, C_OUT + 2] = w1h
    WE[256, C_OUT + 2] = c1h

    XT1h = XT1.astype(np.float16)
    WEh = WE.astype(np.float16)
    A16 = np.asarray(adj_matrix, dtype=np.float16)

    in_maps = []
    for c in range(NCORES):
        in_maps.append(
            {
                "xt1": XT1h,
                "xt1l": np.ascontiguousarray(XT1h[:, c * ROWS : (c + 1) * ROWS]),
                "wext": WEh,
                "adjt": np.ascontiguousarray(
                    A16[c * ROWS : (c + 1) * ROWS, :].T
                ),
            }
        )
    return in_maps


def _run(in_maps, trace=False, b_zero=True):
    key = f"nc_b{int(b_zero)}"
    if key not in _CACHE:
        _CACHE[key] = _build_nc(b_zero=b_zero)
    nc = _CACHE[key]
    res = run_bass_kernel_spmd(
        nc, in_maps, core_ids=list(range(NCORES)), trace=trace
    )
    full = np.concatenate(
        [res.results[c]["out"] for c in range(NCORES)], axis=0
    ).astype(np.float32)
    return full, res


def kernel(node_feats, adj_matrix, W, b, v0, v1):
    in_maps = _prep_inputs(node_feats, adj_matrix, W, b, v0, v1)
    trace = bool(int(os.environ.get("GAT_TRACE", "0")))
    b_zero = not bool(np.any(np.asarray(b)))
    full, _ = _run(in_maps, trace=trace, b_zero=b_zero)
    return full
